# revision 18
# baseline (speedup 1.0000x reference)
"""Bass/Tile TRN2 kernel for nn_MultiHeadSeqAttention_82789789597729.

Math: the reference's softmax / positional scores are dead code -- its output
is exactly  out = concat_h(q_h @ k_h^T @ v_h) @ Wo^T  with no nonlinearity.
By associativity  q (k^T v)  replaces the [M,M] score matrix with a [D,D]
one, collapsing ~69 GFLOP to ~26 GFLOP.

Sharding: tensor-parallel over heads (4 heads / core) x data-parallel over
batch (B=2) -> 8 cores. Each core computes a full-M partial output for its
head group; the host sums the 4 partials per batch (row-parallel unshard).

Schedule (v3):
- PE warm-up dummies (uninitialized operand, result never read) keep the
  tensor engine busy and its DVFS p-state ramping while input DMA lands.
- Inputs land as one SBUF tile per DMA transfer (the tile framework
  coarsens sub-tile write deps to the whole tile), issued in strict
  consumption-order waves over 4 queues so the critical first 1.5 MB is
  never queued behind bulk traffic.
- A = v^T k accumulates into a held PSUM tile, 4 contiguous per-head
  chains (interleaving chains with other matmul groups breaks PSUM
  accumulation on HW). Odd heads sit on PSUM partitions 64-127 so the
  blockdiag cast for the packed C stage stays same-partition.
- C packs 2 heads per matmul (blockdiag A pair vs stacked Wo rows).
- PSUM->SBUF casts alternate Vector / Scalar(Activation) engines.
- P2 (q-proj) for mc=0 is hoisted between P1 and A to hide cast latency;
  each O block runs one mc behind P2.
"""

import numpy as np

import concourse.bass as bass
import concourse.mybir as mybir
import concourse.tile as tile
from concourse.bass_utils import run_bass_kernel_spmd
from concourse.vector_clock import ScopedClock
import bass_rust

B, M, H, K, D = 2, 2048, 1024, 16, 64
N_CORES = 8
HPC = 4           # heads per core
CC = HPC * D      # 256 local feature columns per core
P = 128
N_WARM = 8        # PE warm-up matmuls (512 rows each)
FILLERS = (5, 4, 3, 2, 1, 1)  # extra dummies after early P1 groups

HC_CHUNKS = ((0, 256), (256, 512), (512, 1024), (1024, 2048))
H_CHUNKS = ((0, 1024), (1024, 2048))


# --- workaround: this walrus rejects multi-wait Drain instructions, so split
# --- the TileContext exit drain into one single-wait drain per proc.
def _split_drain_and_barrier(self, tick_clock, wait_clock):
    n_procs = len(list(tick_clock.global_clock))
    for p, t in enumerate(tick_clock.global_clock):
        if t <= 0:
            continue
        single = bass_rust.VectorClock(
            [t if i == p else 0 for i in range(n_procs)]
        )
        d = self.nc.sync.drain()
        wait_clock.add_sem_waits(d.ins, ScopedClock({None: single}))
    self.nc.all_engine_barrier()
    popped = self.nc._tile_sem_poison_stack.pop()
    assert popped is self._sem_poison
    self.nc.clear_and_free_semaphores(list(self.sems.allocated().values()))
    self.nc.all_engine_barrier()


# --- workaround: the same walrus caps sync waits at 1 per instruction
# --- (2 for EventSemaphore). Tile's wait-assignment can attach more; hoist
# --- the extras onto single-wait nop carriers emitted just before.
_ORIG_COMMIT_AND_LOWER = tile.TileContext._commit_and_lower


def _wait_split_commit_and_lower(self, inst, original_block, old_bb_map,
                                 bb_to_exit_bb):
    si = inst.sync_info
    cap = 2 if isinstance(inst, mybir.InstEventSemaphore) else 1
    ow = list(si.on_wait) if si is not None and si.on_wait else []
    if len(ow) > cap and inst.is_executable():
        for w in ow[:-cap]:
            carrier = self.nc.engines[inst.engine].nop(nofuse=True)
            carrier.ins.sync_info = bass_rust.SyncInfo(
                on_wait=[w], on_update=[]
            )
        inst.sync_info = bass_rust.SyncInfo(
            on_wait=ow[-cap:], on_update=list(si.on_update or [])
        )
    return _ORIG_COMMIT_AND_LOWER(
        self, inst, original_block, old_bb_map, bb_to_exit_bb
    )


if not getattr(tile.TileContext, "_split_drain_patched", False):
    tile.TileContext._drain_and_barrier = _split_drain_and_barrier
    tile.TileContext._commit_and_lower = _wait_split_commit_and_lower
    tile.TileContext._split_drain_patched = True


def _build_nc():
    io_dt = mybir.dt.float16
    out_dt = mybir.dt.float16
    f32 = mybir.dt.float32

    nc = bass.Bass()
    hT = nc.dram_tensor("hT", [H, M], io_dt, kind="ExternalInput")
    hcT = nc.dram_tensor("hcT", [H, M], io_dt, kind="ExternalInput")
    wqT = nc.dram_tensor("wqT", [H, CC], io_dt, kind="ExternalInput")
    wkvT = nc.dram_tensor("wkvT", [H, 2 * CC], io_dt, kind="ExternalInput")
    woT = nc.dram_tensor("woT", [CC, H], io_dt, kind="ExternalInput")
    outp = nc.dram_tensor("out", [M, H], out_dt, kind="ExternalOutput")

    IT = H // P           # 8 contraction tiles over feature dim
    LT = M // P           # 16 tiles over sequence dim
    MC = M // 512         # 4 moving chunks over sequence dim
    DT = CC // P          # 2 partition tiles over local feature cols
    JC = H // 512         # 2 chunks over output feature dim

    with tile.TileContext(nc) as tc:
        with (
            tc.tile_pool(name="wp", bufs=1) as wp,
            tc.tile_pool(name="xp", bufs=16) as xp,
            tc.tile_pool(name="big", bufs=1) as big,
            tc.tile_pool(name="op", bufs=4) as op,
            tc.tile_pool(name="ps", bufs=3, space="PSUM") as ps,
            tc.tile_pool(name="po", bufs=4, space="PSUM") as po,
            tc.tile_pool(name="pa", bufs=1, space="PSUM") as pa,
        ):
            warm_w = wp.tile([P, 512], io_dt, tag="warm")
            wkv_sb = wp.tile([P, IT, 2 * CC], io_dt, tag="wkv")
            wq_sb = wp.tile([P, IT, CC], io_dt, tag="wq")
            wo_sb = wp.tile([P, 2, H], io_dt, tag="wo")
            at_sb = wp.tile([P, 2, P], io_dt, tag="at")
            # one tile per DMA transfer: whole-tile dep == transfer dep
            hc_c = [
                [xp.tile([P, c1 - c0], io_dt, tag=f"hc{ci}",
                         name=f"hc_{it}_{ci}")
                 for ci, (c0, c1) in enumerate(HC_CHUNKS)]
                for it in range(IT)
            ]
            h_c = [
                [xp.tile([P, c1 - c0], io_dt, tag=f"h{ci}",
                         name=f"h_{it}_{ci}")
                 for ci, (c0, c1) in enumerate(H_CHUNKS)]
                for it in range(IT)
            ]
            kv_sb = big.tile([P, LT, 2 * CC], io_dt, tag="kv")
            q_sb = big.tile([P, DT, M], io_dt, tag="q")
            c_sb = big.tile([P, DT, H], io_dt, tag="c")

            # --- PE warm-up; result never read. Vector is otherwise idle
            # until its first cast, so it zeroes the operand.
            nc.vector.memset(warm_w[:], 0.0)
            wps = po.tile([P, 512], f32, tag="po")
            for _ in range(N_WARM):
                nc.tensor.matmul(
                    wps[:], warm_w[:, 0:P], warm_w[:], start=True, stop=True
                )

            # --- input DMA in strict consumption-order waves.
            # wave 1+2 rotate over 4 queues; bulk waves go to sync/gpsimd
            # so the scalar/vector queues stay clear for casts.
            wkv_r = wkvT.rearrange("(it p) c -> p it c", p=P)
            wq_r = wqT.rearrange("(it p) c -> p it c", p=P)
            all4 = [nc.sync, nc.gpsimd, nc.scalar]
            two = [nc.sync, nc.gpsimd]
            n4 = 0
            n2 = 0

            def dma4(out, in_):
                nonlocal n4
                all4[n4 % 3].dma_start(out=out, in_=in_)
                n4 += 1

            def dma2(out, in_):
                nonlocal n2
                two[n2 % 2].dma_start(out=out, in_=in_)
                n2 += 1

            # wave 1: hc cols 0:256 interleaved with wkv (P1 lt=0..1)
            for it in range(IT):
                c0, c1 = HC_CHUNKS[0]
                dma4(hc_c[it][0][:], hcT[it * P:(it + 1) * P, c0:c1])
                dma4(wkv_sb[:, it, :], wkv_r[:, it, :])
            # wave 2: hc cols 256:512 (lt=2..3)
            for it in range(IT):
                c0, c1 = HC_CHUNKS[1]
                dma4(hc_c[it][1][:], hcT[it * P:(it + 1) * P, c0:c1])
            # waves 3-4: hc cols 512:1024, 1024:2048
            for ci in (2, 3):
                c0, c1 = HC_CHUNKS[ci]
                for it in range(IT):
                    dma4(hc_c[it][ci][:], hcT[it * P:(it + 1) * P, c0:c1])
            # wave 5: wq, wo
            dma2(wq_sb[:, 0:4, :], wq_r[:, 0:4, :])
            dma2(wq_sb[:, 4:8, :], wq_r[:, 4:8, :])
            dma2(wo_sb[:], woT.rearrange("(pp p) j -> p pp j", p=P))
            # wave 6: h
            for ci, (c0, c1) in enumerate(H_CHUNKS):
                for it in range(IT):
                    dma2(h_c[it][ci][:], hT[it * P:(it + 1) * P, c0:c1])

            # blockdiag zeros for at_sb; needed at ~40us, issue now
            nc.gpsimd.memset(at_sb[:], 0.0)

            # cast engines alternate DVE / Activation
            def cast(i, out, in_):
                if i % 2 == 1:
                    nc.scalar.copy(out, in_)
                else:
                    nc.vector.tensor_copy(out, in_)

            def hc_slice(it, lt):
                for ci, (c0, c1) in enumerate(HC_CHUNKS):
                    if c0 <= lt * P < c1:
                        return hc_c[it][ci][:, lt * P - c0:(lt + 1) * P - c0]
                raise AssertionError

            # --- P1: fused k|v projection. The first groups are paced by
            # input DMA; filler dummies keep the PE busy (and its clock
            # ramped) between them instead of idling.
            for lt in range(LT):
                acc = ps.tile([P, 2 * CC], f32, tag="ps")
                for it in range(IT):
                    nc.tensor.matmul(
                        acc[:],
                        hc_slice(it, lt),
                        wkv_sb[:, it, :],
                        start=(it == 0), stop=(it == IT - 1),
                    )
                cast(lt, kv_sb[:, lt, :], acc[:])
                if lt < len(FILLERS):
                    for _ in range(FILLERS[lt]):
                        nc.tensor.matmul(
                            wps[:], warm_w[:, 0:P], warm_w[:],
                            start=True, stop=True,
                        )

            # --- P2: q projection for one 512-column chunk of m
            def p2(mc):
                ci = 0 if mc < 2 else 1
                c0 = H_CHUNKS[ci][0]
                for dt_i in range(DT):
                    acc = ps.tile([P, 512], f32, tag="ps")
                    for it in range(IT):
                        nc.tensor.matmul(
                            acc[:],
                            wq_sb[:, it, dt_i * P:(dt_i + 1) * P],
                            h_c[it][ci][:, mc * 512 - c0:(mc + 1) * 512 - c0],
                            start=(it == 0), stop=(it == IT - 1),
                        )
                    cast(dt_i, q_sb[:, dt_i, mc * 512:(mc + 1) * 512], acc[:])

            p2(0)

            # --- A: held PSUM accumulator, 4 contiguous per-head chains;
            # odd heads on partitions 64..127 for the blockdiag layout.
            a_ps = pa.tile([P, 2, D], f32, tag="pa")
            for hh in range(HPC):
                po_h = (hh % 2) * D
                for lt in range(LT):
                    nc.tensor.matmul(
                        a_ps[po_h:po_h + D, hh // 2, :],
                        kv_sb[:, lt, CC + hh * D:CC + (hh + 1) * D],
                        kv_sb[:, lt, hh * D:(hh + 1) * D],
                        start=(lt == 0), stop=(lt == LT - 1),
                    )
            for hh in range(HPC):
                po_h = (hh % 2) * D
                cast(hh, at_sb[po_h:po_h + D, hh // 2, po_h:po_h + D],
                     a_ps[po_h:po_h + D, hh // 2, :])

            # spacer: the A->at_sb casts run on V/S while p2(1) computes
            p2(1)

            # --- C: rows of (A_h Wo_h^T), packed 2 heads per matmul
            nct = 0
            for pp in range(2):
                for jc in range(JC):
                    acc = po.tile([P, 512], f32, tag="po")
                    nc.tensor.matmul(
                        acc[:],
                        at_sb[:, pp, :],
                        wo_sb[:, pp, jc * 512:(jc + 1) * 512],
                        start=True, stop=True,
                    )
                    cast(nct, c_sb[:, pp, jc * 512:(jc + 1) * 512], acc[:])
                    nct += 1

            # --- O for a block of 4 mt, one mc behind P2
            n_out = 0

            def o_block(mc):
                nonlocal n_out
                for mt in range(mc * 4, (mc + 1) * 4):
                    o_t = op.tile([P, H], out_dt, tag="o")
                    for jc in range(JC):
                        acc = po.tile([P, 512], f32, tag="po")
                        for dt_i in range(DT):
                            nc.tensor.matmul(
                                acc[:],
                                q_sb[:, dt_i, mt * P:(mt + 1) * P],
                                c_sb[:, dt_i, jc * 512:(jc + 1) * 512],
                                start=(dt_i == 0), stop=(dt_i == DT - 1),
                            )
                        cast(mt * 2 + jc, o_t[:, jc * 512:(jc + 1) * 512],
                             acc[:])
                        all4[n_out % 3].dma_start(
                            out=outp[mt * P:(mt + 1) * P,
                                     jc * 512:(jc + 1) * 512],
                            in_=o_t[:, jc * 512:(jc + 1) * 512],
                        )
                        n_out += 1

            p2(2)
            o_block(0)
            p2(3)
            o_block(1)
            o_block(2)
            o_block(3)

    return nc


_NC_CACHE = {}


def _get_nc():
    if "nc" not in _NC_CACHE:
        _NC_CACHE["nc"] = _build_nc()
    return _NC_CACHE["nc"]


def _cast(a):
    return np.ascontiguousarray(a).astype(np.float16)


def make_in_maps(h, h_cache, Wq, Wk, Wv, Wo):
    in_maps = []
    for c in range(N_CORES):
        b, g = divmod(c, 4)
        cols = slice(g * CC, (g + 1) * CC)
        in_maps.append({
            "hT": _cast(h[b].T),
            "hcT": _cast(h_cache[b].T),
            "wqT": _cast(Wq[cols, :].T),
            "wkvT": _cast(np.concatenate(
                [Wk[cols, :].T, Wv[cols, :].T], axis=1)),
            "woT": _cast(Wo[:, cols].T),
        })
    return in_maps


def kernel(h, h_cache, key_pe, Wq, Wk, Wv, Wo, _bass_results=None):
    h = np.asarray(h)
    h_cache = np.asarray(h_cache)
    Wq, Wk, Wv, Wo = (np.asarray(a) for a in (Wq, Wk, Wv, Wo))
    nc = _get_nc()
    in_maps = make_in_maps(h, h_cache, Wq, Wk, Wv, Wo)
    res = run_bass_kernel_spmd(nc, in_maps, list(range(N_CORES)))
    if _bass_results is not None:
        _bass_results.append(res)
    out = np.zeros((B, M, H), np.float32)
    for c in range(N_CORES):
        out[c // 4] += res.results[c]["out"].astype(np.float32)
    return out
